# revision 62
# baseline (speedup 1.0000x reference)
"""AngleScorerEnergy Trainium2 kernel: MoE-style routing of residue rows to
per-amino-acid RealNVP flow experts, expert-parallel across 8 NeuronCores.

Host: gather rows, sort by aa, pack into 8 cores x 3 uniform slots (one graph,
per-slot weights are input data). Omega flow is a closed-form affine (its
coupling MLPs always see zero input), folded into per-slot scalars.
Device: bb (nfea=2) + sc (nfea=5) flows; s/t nets packed into block-diagonal
128x128 matmuls; logdet/z^2 reduction via a [15,4] matmul.
"""

import math
import numpy as np

H = 64
L = 6
N_AA = 20
NFB = 2   # bb features (phi/psi)
NFS = 5   # sc features (chi1..chi5)
N_CHI = (0, 5, 2, 2, 1, 3, 3, 0, 2, 1, 2, 4, 3, 2, 0, 1, 1, 2, 2, 1)
LOG2PI = float(np.log(2.0 * np.pi))
N_CORES = 8
N_SLOTS = 3
TILE_N = 512
CHUNK = 496      # cols 496..511 of the PSUM accumulators host dummy writes
USE_LRELU = True


def _lrelu(x):
    return np.where(x > 0, x, 0.01 * x)


def _om_affine(p, aa):
    """Omega flow closed form: masked input is always 0 for nfea=1, so each
    effective layer (l=4,2,0) applies z -> (z - t_l) * exp(-s_l) with constant
    s_l, t_l = MLP(0). Returns (E, D, S) with z_fin = E*z - D, logdet = -S."""
    E = np.float64(1.0)
    D = np.float64(0.0)
    S = np.float64(0.0)
    for l in (4, 2, 0):  # applied in this order (reversed(range(6)), skipping odd)
        h1 = _lrelu(p["sb1"][aa, l].astype(np.float64))
        h2 = _lrelu(p["sW2"][aa, l].astype(np.float64) @ h1 + p["sb2"][aa, l].astype(np.float64))
        s = math.tanh(float(p["sW3"][aa, l, 0].astype(np.float64) @ h2 + p["sb3"][aa, l, 0]))
        h1t = _lrelu(p["tb1"][aa, l].astype(np.float64))
        h2t = _lrelu(p["tW2"][aa, l].astype(np.float64) @ h1t + p["tb2"][aa, l].astype(np.float64))
        t = float(p["tW3"][aa, l, 0].astype(np.float64) @ h2t + p["tb3"][aa, l, 0])
        a = math.exp(-s)
        # z <- (z - t) * a applied after existing affine z = E*z - D:
        # new z = (E*z - D - t) * a = (a*E)*z - a*(D + t)
        E = a * E
        D = a * (D + t)
        S = S + s
    return float(E), float(D), float(S)


def _pack_slot_weights(bb_p, sc_p, aa, nchi):
    """Pack one slot's device weight arrays (mask folded into W1)."""
    out = {}
    for fl, pp, nf in (("bb", bb_p, NFB), ("sc", sc_p, NFS)):
        nuse = nf if fl == "bb" else nchi
        l1 = np.zeros((nf, L * 128), np.float32)
        l2 = np.zeros((128, L * 128), np.float32)
        l3s = np.zeros((128, L * nf), np.float32)
        l3t = np.zeros((128, L * nf), np.float32)
        b12 = np.zeros((128, 2 * L), np.float32)   # cols 0..5 b1, 6..11 b2
        b3s = np.zeros((nf, L), np.float32)
        idb = np.zeros((nf + 1, L * nf), np.float32)
        for l in range(L):
            m = np.array([(l + j) % 2 for j in range(nf)], np.float32)
            keep = (np.arange(nf) < nuse).astype(np.float32)
            # L1 lhsT [nf, 128]: cols 0-63 sW1.T, 64-127 tW1.T; input mask m
            # folded (z_ = m*z) plus chi padding
            fold_in = (m * keep)[:, None]
            l1[:, l * 128:l * 128 + 64] = pp["sW1"][aa, l, :, :nf].T * fold_in
            l1[:, l * 128 + 64:(l + 1) * 128] = pp["tW1"][aa, l, :, :nf].T * fold_in
            # L2 block diag
            l2[0:64, l * 128:l * 128 + 64] = pp["sW2"][aa, l].T
            l2[64:128, l * 128 + 64:(l + 1) * 128] = pp["tW2"][aa, l].T
            # L3 lhsT split into s-net [128, nf] and t-net [128, nf] with the
            # output mask (1-m) folded: masked features get s=0, t=0 so the
            # coupling update (z - t)*exp(-s) leaves them unchanged.
            fold_out = ((1.0 - m) * keep)[None, :]
            l3s[0:64, l * nf:(l + 1) * nf] = pp["sW3"][aa, l, :nf].T * fold_out
            # t-path negated: PSUM accumulates (z - t) directly (with +z via
            # an identity matmul in the graph)
            l3t[64:128, l * nf:(l + 1) * nf] = -pp["tW3"][aa, l, :nf].T * fold_out
            b3s[:, l] = pp["sb3"][aa, l, :nf] * fold_out[0]
            idb[0:nf, l * nf:(l + 1) * nf] = np.eye(nf, dtype=np.float32)
            idb[nf, l * nf:(l + 1) * nf] = -pp["tb3"][aa, l, :nf] * fold_out[0]
            b12[0:64, l] = pp["sb1"][aa, l]
            b12[64:128, l] = pp["tb1"][aa, l]
            b12[0:64, L + l] = pp["sb2"][aa, l]
            b12[64:128, L + l] = pp["tb2"][aa, l]
        out[f"l1{fl}"] = l1
        out[f"l2{fl}"] = l2
        out[f"l3s{fl}"] = l3s
        out[f"l3t{fl}"] = l3t
        out[f"b{fl}"] = b12
        out[f"b3s{fl}"] = b3s
        out[f"idb{fl}"] = idb
    return out


# single consolidated weight bank: name -> (row_count, col_offset, col_count)
_WSHAPES = [
    ("l1bb", NFB, L * 128), ("l1sc", NFS, L * 128),
    ("l2bb", 128, L * 128), ("l2sc", 128, L * 128),
    ("l3sbb", 128, L * NFB), ("l3tbb", 128, L * NFB),
    ("l3ssc", 128, L * NFS), ("l3tsc", 128, L * NFS),
    ("bbb", 128, 2 * L), ("bsc", 128, 2 * L),
    ("b3sbb", NFB, L), ("b3ssc", NFS, L),
    ("idbbb", NFB + 1, L * NFB), ("idbsc", NFS + 1, L * NFS),
    ("misc", 8, 12),
]
_WOFF = {}
_WCOL = 0
for _n, _p, _c in _WSHAPES:
    _WOFF[_n] = (_p, _WCOL, _c)
    _WCOL += _c


def _route(row_aa):
    """Split 20 aa groups into N_CORES*N_SLOTS shards, assign to (core, slot).
    Returns slot capacities C[j] and per-(core,slot) global row-id arrays."""
    nrows = row_aa.shape[0]
    order = np.argsort(row_aa, kind="stable")
    counts = np.bincount(row_aa, minlength=N_AA)
    starts = np.zeros(N_AA + 1, np.int64)
    starts[1:] = np.cumsum(counts)
    shards = [(int(counts[a]), int(a), int(starts[a]), int(counts[a]))
              for a in range(N_AA)]  # (size, aa, start, size)
    # split largest until we have N_CORES*N_SLOTS shards
    import heapq
    heap = [(-s, a, st, s) for (s, a, st, s2) in shards if s > 0]
    if not heap:
        heap = [(0, 0, 0, 0)]
    heapq.heapify(heap)
    while len(heap) < N_CORES * N_SLOTS:
        negs, a, st, s = heapq.heappop(heap)
        h1 = (s + 1) // 2
        heapq.heappush(heap, (-h1, a, st, h1))
        heapq.heappush(heap, (-(s - h1), a, st + h1, s - h1))
    shards = sorted([(-n, a, st, s) for (n, a, st, s) in heap], reverse=True)
    shards = [(s, a, st) for (_, a, st, s) in shards]
    caps = []
    assign = [[None] * N_SLOTS for _ in range(N_CORES)]
    for j in range(N_SLOTS):
        grp = shards[j * N_CORES:(j + 1) * N_CORES]
        caps.append(grp[0][0])
        for c in range(N_CORES):
            s, a, st = grp[c]
            assign[c][j] = (a, order[st:st + s])
    return caps, assign


def _split_multi_waits(nc, mybir):
    """Walrus codegen allows only ONE sync-wait per PE/ACT/DVE/Pool
    instruction; hoist extra waits into preceding same-engine NoOps."""
    for fn in nc.m.functions:
        for blk in fn.blocks:
            new = []
            for inst in blk.instructions:
                si = getattr(inst, "sync_info", None)
                eng = getattr(inst, "engine", None)
                if si is not None and si.on_wait and len(si.on_wait) > 1:
                    waits = list(si.on_wait)
                    for w in waits[:-1]:
                        nop = mybir.InstNoOp(
                            name=nc.get_next_instruction_name(),
                            engine=eng,
                            sync_info=mybir.SyncInfo(on_wait=[w], on_update=[]),
                            bass_nofuse=True,
                        )
                        new.append(nop)
                    inst.sync_info = mybir.SyncInfo(on_wait=[waits[-1]],
                                                    on_update=si.on_update)
                new.append(inst)
            blk.instructions = new


def _build_graph(caps, ntiles, split_waits=True, use_lrelu=USE_LRELU):
    """SPMD Bass graph. The (up to 2) row-chunks of each slot are processed as
    one "pair" sharing weights: their matmuls write the two 512-col halves of
    shared 2-bank PSUM tiles, so every ACT/DVE op covers both chunks at once.
    Ops are emitted stage-major across (pair, flow) chains to keep engine
    queues filled with independent work."""
    import concourse.bass as bass
    import concourse.mybir as mybir
    from concourse import tile

    f32 = mybir.dt.float32
    nc = bass.Bass()

    prm = {}
    for j in range(N_SLOTS):
        C = caps[j]
        if C == 0:
            continue
        prm[f"xbb{j}"] = nc.declare_dram_parameter(f"xbb{j}", [NFB + 1, C], f32, isOutput=False)
        prm[f"xom{j}"] = nc.declare_dram_parameter(f"xom{j}", [1, C], f32, isOutput=False)
        prm[f"xsc{j}"] = nc.declare_dram_parameter(f"xsc{j}", [NFS + 1, C], f32, isOutput=False)
        prm[f"out{j}"] = nc.declare_dram_parameter(f"out{j}", [C], f32, isOutput=True)
    prm["wbank"] = nc.declare_dram_parameter("wbank", [N_SLOTS, 128, _WCOL], f32,
                                             isOutput=False)
    AF = mybir.ActivationFunctionType
    W2 = 2 * TILE_N   # paired tile width

    # per-slot chunk lists: [(o0, o1), ...] (<= 2 per slot by construction)
    pairs = []
    for j in range(N_SLOTS):
        C = caps[j]
        if C == 0:
            continue
        nch = max(1, -(-C // CHUNK))
        offs = np.linspace(0, C, nch + 1).astype(int)
        ch = [(int(offs[t]), int(offs[t + 1])) for t in range(nch)
              if offs[t + 1] > offs[t]]
        for k in range(0, len(ch), 2):
            pr = ch[k:k + 2]
            if len(pr) == 2:
                # equalize widths inside a pair by overlapping backwards
                # (overlapped rows recompute identical values - benign)
                nt = max(pr[0][1] - pr[0][0], pr[1][1] - pr[1][0])
                pr = [(pr[0][0], pr[0][0] + nt), (pr[1][1] - nt, pr[1][1])]
            pairs.append((j, pr))

    with tile.TileContext(nc) as tc:
        with (
            tc.tile_pool(name="wpool", bufs=2) as wpool,
            tc.tile_pool(name="zpool", bufs=2) as zpool,
            tc.tile_pool(name="hpool", bufs=3) as hpool,
            tc.tile_pool(name="upool", bufs=2) as upool,
            tc.tile_pool(name="z2pool", bufs=2) as z2pool,
            tc.tile_pool(name="spool", bufs=2) as spool,
            tc.tile_pool(name="gpool", bufs=4) as gpool,
            tc.tile_pool(name="cpool", bufs=1) as cpool,
            tc.tile_pool(name="pp", bufs=2, space="PSUM") as pp,
            tc.tile_pool(name="po3", bufs=2, space="PSUM") as po3,
        ):
            slot_wt = {}

            def halves(c):
                return [(h * TILE_N, o1 - o0) for h, (o0, o1) in enumerate(c["chunks"])]

            def wide(c, t, P=None):
                # AP covering both halves: strided free dim [P, 2, nt]
                nts = [nt for _, nt in halves(c)]
                mx = max(nts)
                p = t.shape[0] if P is None else P
                if len(nts) == 1:
                    return t[0:p, 0:mx]
                return t[0:p, 0:2 * TILE_N].rearrange("p (g n) -> p g n", g=2)[:, :, 0:mx]

            FL = [("bb", NFB), ("sc", NFS)]
            for j, chunks in pairs:
                if j not in slot_wt:
                    wtile = wpool.tile([128, _WCOL], f32, tag="wbank")
                    nc.sync.dma_start(wtile[:], prm["wbank"][j])
                    slot_wt[j] = {n: wtile[0:p, c0:c0 + cn]
                                  for n, (p, c0, cn) in _WOFF.items()}
                wt = slot_wt[j]
                c = dict(j=j, chunks=chunks, wt=wt, misc=wt["misc"])
                c["zbb"] = zpool.tile([NFB + 1, W2], f32, tag="zbb", name="zbb")
                c["zom"] = zpool.tile([1, W2], f32, tag="zom", name="zom")
                c["zsc"] = zpool.tile([NFS + 1, W2], f32, tag="zsc", name="zsc")
                c["Sbb"] = spool.tile([NFB, W2], f32, tag="Sbb", name="Sbb")
                c["Ssc"] = spool.tile([NFS, W2], f32, tag="Ssc", name="Ssc")
                for h, (o0, o1) in enumerate(chunks):
                    nt = o1 - o0
                    hb = h * TILE_N
                    nc.sync.dma_start(c["zbb"][:, hb:hb + nt], prm[f"xbb{j}"][:, o0:o1])
                    nc.sync.dma_start(c["zom"][:, hb:hb + nt], prm[f"xom{j}"][:, o0:o1])
                    nc.sync.dma_start(c["zsc"][:, hb:hb + nt], prm[f"xsc{j}"][:, o0:o1])
                c["first"] = {"bb": True, "sc": True}

                for l in range(L - 1, -1, -1):
                    for fl, nf in FL:
                        zt = c[f"z{fl}"]
                        P1 = pp.tile([128, W2], f32, tag="P", name="P1")
                        for hb, nt in halves(c):
                            nc.tensor.matmul(P1[:, hb:hb + nt],
                                             c["wt"][f"l1{fl}"][:, l * 128:(l + 1) * 128],
                                             zt[0:nf, hb:hb + nt])
                        c[f"P{fl}"] = P1
                    for fl, nf in FL:
                        y1 = hpool.tile([128, W2], f32, tag="y", name="y1")
                        bias = c["wt"][f"b{fl}"][:, l:l + 1]
                        if use_lrelu:
                            nc.scalar.activation(wide(c, y1), wide(c, c[f"P{fl}"]),
                                                 AF.Lrelu, bias=bias, alpha=0.01)
                        else:
                            nc.scalar.activation(wide(c, y1), wide(c, c[f"P{fl}"]),
                                                 AF.Identity, bias=bias)
                            u1 = upool.tile([128, W2], f32, tag="u", name="u1")
                            nc.vector.tensor_scalar_mul(wide(c, u1), wide(c, y1), 0.01)
                            nc.vector.tensor_max(wide(c, y1), wide(c, y1), wide(c, u1))
                        c[f"y{fl}"] = y1
                    for fl, nf in FL:
                        P2 = pp.tile([128, W2], f32, tag="P", name="P2")
                        for hb, nt in halves(c):
                            nc.tensor.matmul(P2[:, hb:hb + nt],
                                             c["wt"][f"l2{fl}"][:, l * 128:(l + 1) * 128],
                                             c[f"y{fl}"][:, hb:hb + nt])
                        c[f"P{fl}"] = P2
                    for fl, nf in FL:
                        y2 = hpool.tile([128, W2], f32, tag="y", name="y2")
                        bias = c["wt"][f"b{fl}"][:, L + l:L + l + 1]
                        if use_lrelu:
                            nc.scalar.activation(wide(c, y2), wide(c, c[f"P{fl}"]),
                                                 AF.Lrelu, bias=bias, alpha=0.01)
                        else:
                            nc.scalar.activation(wide(c, y2), wide(c, c[f"P{fl}"]),
                                                 AF.Identity, bias=bias)
                            u2 = upool.tile([128, W2], f32, tag="u", name="u2")
                            nc.vector.tensor_scalar_mul(wide(c, u2), wide(c, y2), 0.01)
                            nc.vector.tensor_max(wide(c, y2), wide(c, y2), wide(c, u2))
                        c[f"y{fl}"] = y2
                    for fl, nf in FL:
                        wt_ = c["wt"]
                        zt = c[f"z{fl}"]
                        Os = po3.tile([nf, W2], f32, tag="O3", name="Os")
                        Ot = po3.tile([nf, W2], f32, tag="O3", name="Ot")
                        for hb, nt in halves(c):
                            nc.tensor.matmul(Os[:, hb:hb + nt],
                                             wt_[f"l3s{fl}"][:, l * nf:(l + 1) * nf],
                                             c[f"y{fl}"][:, hb:hb + nt],
                                             start=True, stop=True)
                            # Ot = z - t (t weights negated; idb adds z and
                            # -b3t via the z tile's trailing ones row)
                            nc.tensor.matmul(Ot[:, hb:hb + nt],
                                             wt_[f"l3t{fl}"][:, l * nf:(l + 1) * nf],
                                             c[f"y{fl}"][:, hb:hb + nt],
                                             start=True, stop=False)
                            nc.tensor.matmul(Ot[:, hb:hb + nt],
                                             wt_[f"idb{fl}"][:, l * nf:(l + 1) * nf],
                                             zt[:, hb:hb + nt], start=False, stop=True)
                        c[f"Os{fl}"] = Os
                        c[f"Ot{fl}"] = Ot
                    for fl, nf in FL:
                        st_ = spool.tile([nf, W2], f32, tag=f"s{fl}", name="st")
                        nc.scalar.activation(wide(c, st_), wide(c, c[f"Os{fl}"]), AF.Tanh,
                                             bias=c["wt"][f"b3s{fl}"][:, l:l + 1])
                        c[f"s{fl}"] = st_
                    for fl, nf in FL:
                        acc = c["Sbb"] if fl == "bb" else c["Ssc"]
                        if c["first"][fl]:
                            nc.vector.tensor_copy(wide(c, acc), wide(c, c[f"s{fl}"]))
                            c["first"][fl] = False
                        else:
                            nc.vector.tensor_add(wide(c, acc), wide(c, acc),
                                                 wide(c, c[f"s{fl}"]))
                    for fl, nf in FL:
                        et = spool.tile([nf, W2], f32, tag=f"e{fl}", name="et")
                        nc.scalar.activation(wide(c, et), wide(c, c[f"s{fl}"]),
                                             AF.Exp, scale=-1.0)
                        c[f"e{fl}"] = et
                    for fl, nf in FL:
                        nc.vector.tensor_mul(wide(c, c[f"z{fl}"], nf),
                                             wide(c, c[f"Ot{fl}"]),
                                             wide(c, c[f"e{fl}"]))

                # tail
                misc = c["misc"]
                z2bb = z2pool.tile([NFB, W2], f32, tag="z2bb", name="z2bb")
                z2om = z2pool.tile([1, W2], f32, tag="z2om", name="z2om")
                z2sc = z2pool.tile([NFS, W2], f32, tag="z2sc", name="z2sc")
                nc.scalar.activation(wide(c, z2bb), wide(c, c["zbb"], NFB), AF.Square)
                nc.scalar.activation(wide(c, z2om), wide(c, c["zom"]), AF.Square,
                                     scale=misc[0:1, 8:9], bias=misc[0:1, 9:10])
                nc.scalar.activation(wide(c, z2sc), wide(c, c["zsc"], NFS), AF.Square)
                Wb = po3.tile([1, W2], f32, tag="O3", name="Wb")
                Ws = po3.tile([1, W2], f32, tag="O3", name="Ws")
                for hb, nt in halves(c):
                    nc.tensor.matmul(Wb[0:1, hb:hb + nt], misc[0:NFB, 0:1],
                                     c["Sbb"][:, hb:hb + nt],
                                     start=True, stop=False, skip_group_check=True)
                    nc.tensor.matmul(Wb[0:1, hb:hb + nt], misc[0:NFB, 2:3],
                                     z2bb[:, hb:hb + nt],
                                     start=False, stop=True, skip_group_check=True)
                    nc.tensor.matmul(Ws[0:1, hb:hb + nt], misc[0:NFS, 1:2],
                                     c["Ssc"][:, hb:hb + nt],
                                     start=True, stop=False, skip_group_check=True)
                    nc.tensor.matmul(Ws[0:1, hb:hb + nt], misc[0:NFS, 3:4],
                                     z2sc[:, hb:hb + nt],
                                     start=False, stop=True, skip_group_check=True)
                nc.vector.tensor_scalar_mul(wide(c, z2om), wide(c, z2om),
                                            misc[0:1, 4:5])
                Vb = gpool.tile([1, W2], f32, tag="V", name="Vb")
                Vs = gpool.tile([1, W2], f32, tag="V", name="Vs")
                nc.vector.tensor_scalar(wide(c, Vb), wide(c, Wb),
                                        misc[0:1, 5:6], 5.0,
                                        mybir.AluOpType.add, mybir.AluOpType.min)
                nc.vector.tensor_scalar(wide(c, Vs), wide(c, Ws),
                                        misc[0:1, 6:7], 5.0,
                                        mybir.AluOpType.add, mybir.AluOpType.min)
                nc.vector.tensor_add(wide(c, Vb), wide(c, Vb), wide(c, Vs))
                nc.vector.tensor_add(wide(c, Vb), wide(c, Vb), wide(c, z2om))
                G = gpool.tile([1, W2], f32, tag="V", name="G")
                nc.scalar.activation(wide(c, G), wide(c, Vb), AF.Relu,
                                     scale=-1.0, bias=misc[0:1, 7:8])
                nc.vector.tensor_scalar_min(wide(c, G), wide(c, G), 5.0)
                for h, (o0, o1) in enumerate(chunks):
                    hb = h * TILE_N
                    nc.sync.dma_start(prm[f"out{j}"][o0:o1], G[0:1, hb:hb + o1 - o0])
    if split_waits:
        _split_multi_waits(nc, mybir)
    return nc


def kernel(angles, weight_bb, weight_omega, weight_sc,
           bb_params, omega_params, sc_params,
           batch_idx, chain_idx, resnum_idx, resname_idx,
           _emulate=False, _bench=0):
    angles = np.asarray(angles)
    B, Cc, Rr, A, _ = angles.shape
    batch_idx = np.asarray(batch_idx)
    chain_idx = np.asarray(chain_idx)
    resnum_idx = np.asarray(resnum_idx)
    resname_idx = np.asarray(resname_idx)
    bb_params = {k: np.asarray(v) for k, v in bb_params.items()}
    omega_params = {k: np.asarray(v) for k, v in omega_params.items()}
    sc_params = {k: np.asarray(v) for k, v in sc_params.items()}

    x = angles[batch_idx, chain_idx, resnum_idx]         # [N, A, 8]
    N = x.shape[0]
    rows = x.reshape(N * A, 8).astype(np.float32)
    row_aa = np.repeat(np.asarray(resname_idx, np.int64), A)

    w_bb = float(1.0 - np.tanh(-np.asarray(weight_bb)[0]))
    w_om = float(1.0 - np.tanh(-np.asarray(weight_omega)[0]))
    w_sc_all = 1.0 - np.tanh(-np.asarray(weight_sc))

    caps, assign = _route(row_aa)
    ntiles = [max(1, -(-c // CHUNK)) for c in caps]

    # host-side packing per (core, slot)
    in_maps = []
    aa_cache = {}
    for c in range(N_CORES):
        m = {}
        wbank = np.zeros((N_SLOTS, 128, _WCOL), np.float32)

        def put(j_, name, arr):
            p, c0, cn = _WOFF[name]
            wbank[j_, 0:p, c0:c0 + cn] = arr
        for j in range(N_SLOTS):
            C = caps[j]
            if C == 0:
                continue
            aa, rid = assign[c][j]
            nchi = N_CHI[aa]
            if aa not in aa_cache:
                wts = _pack_slot_weights(bb_params, sc_params, aa, nchi)
                E, D, S_om = _om_affine(omega_params, aa)
                aa_cache[aa] = (wts, E, D, S_om)
            wts, E, D, S_om = aa_cache[aa]
            for k, v in wts.items():
                put(j, k, v)
            w_sc = float(w_sc_all[aa])
            keep = (np.arange(NFS) < nchi)
            misc = np.zeros((8, 12), np.float32)
            misc[0:4, 0] = -w_bb
            misc[0:5, 1] = -w_sc
            misc[0:2, 2] = -0.5 * w_bb
            misc[0:5, 3] = np.where(keep, -0.5 * w_sc, 0.0)
            misc[0, 4] = -0.5 * w_om
            misc[0, 5] = -w_bb * LOG2PI                       # cvec_bb
            misc[0, 6] = -0.5 * nchi * LOG2PI * w_sc          # cvec_sc
            misc[0, 7] = -(w_om * (-0.5 * LOG2PI - S_om))     # -cvec_om
            misc[0, 8] = E
            misc[0, 9] = -D
            put(j, "misc", misc)
            # row data, feature-major, zero-padded
            xb = np.zeros((NFB + 1, C), np.float32)
            xo = np.zeros((1, C), np.float32)
            xs = np.zeros((NFS + 1, C), np.float32)
            n = len(rid)
            xb[0:NFB, :n] = rows[rid, 0:2].T
            xb[NFB] = 1.0
            xo[:, :n] = rows[rid, 2:3].T
            xs[:nchi, :n] = rows[rid, 3:3 + nchi].T
            xs[NFS] = 1.0
            m[f"xbb{j}"] = xb
            m[f"xom{j}"] = xo
            m[f"xsc{j}"] = xs
        m["wbank"] = wbank
        in_maps.append(m)

    if _emulate:
        outs = [_emulate_core(in_maps[c], caps, ntiles) for c in range(N_CORES)]
    else:
        nc = _build_graph(caps, ntiles)
        from concourse.bass_utils import run_bass_kernel_spmd
        res = run_bass_kernel_spmd(nc, in_maps, core_ids=list(range(N_CORES)))
        outs = res.results
        if _bench:
            import time as _time
            ts = []
            for _ in range(_bench):
                t0 = _time.perf_counter()
                run_bass_kernel_spmd(nc, in_maps, core_ids=list(range(N_CORES)))
                ts.append(_time.perf_counter() - t0)
            kernel._bench_ns = int(sorted(ts)[len(ts) // 2] * 1e9)

    score_rows = np.zeros(N * A, np.float32)
    for c in range(N_CORES):
        for j in range(N_SLOTS):
            if caps[j] == 0:
                continue
            aa, rid = assign[c][j]
            vals = np.asarray(outs[c][f"out{j}"])
            score_rows[rid] = vals[:len(rid)]

    score_res = score_rows.reshape(N, A)
    out = np.zeros((B, Cc, Rr, A), np.float32)
    out[batch_idx, chain_idx, resnum_idx] = score_res
    return out


def _emulate_core(m, caps, ntiles):
    """Numpy emulation of exactly what the device graph computes (validates
    host packing + graph math without hardware)."""
    out = {}
    for j in range(N_SLOTS):
        C = caps[j]
        if C == 0:
            continue
        zbb = m[f"xbb{j}"][0:NFB].copy()
        zom = m[f"xom{j}"].copy()
        zsc = m[f"xsc{j}"][0:NFS].copy()

        def w(name):
            p, c0, cn = _WOFF[name]
            return m["wbank"][j, 0:p, c0:c0 + cn]
        misc = w("misc")
        acc = {0: np.zeros(C, np.float32), 32: np.zeros(C, np.float32),
               64: np.zeros(C, np.float32)}
        for l in range(L - 1, -1, -1):
            for (fl, nf, zt, wcol, arow) in (
                ("bb", NFB, zbb, 0, 0),
                ("sc", NFS, zsc, 1, 64),
            ):
                l1, l2 = w(f"l1{fl}"), w(f"l2{fl}")
                l3s, l3t = w(f"l3s{fl}"), w(f"l3t{fl}")
                bb_ = w(f"b{fl}")
                b3s, idb = w(f"b3s{fl}"), w(f"idb{fl}")
                P1 = l1[:, l * 128:(l + 1) * 128].T @ zt
                y1 = _lrelu(P1 + bb_[:, l:l + 1])
                P2 = l2[:, l * 128:(l + 1) * 128].T @ y1
                y2 = _lrelu(P2 + bb_[:, L + l:L + l + 1])
                zx = np.concatenate([zt, np.ones((1, zt.shape[1]), np.float32)])
                Os = l3s[:, l * nf:(l + 1) * nf].T @ y2
                # l3t/idb pre-negated: Ot = z - t
                Ot = (l3t[:, l * nf:(l + 1) * nf].T @ y2
                      + idb[:, l * nf:(l + 1) * nf].T @ zx)
                s = np.tanh(Os + b3s[:, l:l + 1])
                acc[arow] += misc[0:nf, wcol] @ s
                e = np.exp(-s)
                zt[:] = Ot * e
        z2om = np.square(misc[0, 8] * zom[0] + misc[0, 9])
        acc[0] += misc[0:NFB, 2] @ np.square(zbb)
        acc[32] += misc[0, 4] * z2om
        acc[64] += misc[0:NFS, 3] @ np.square(zsc)
        Vb = np.minimum(acc[0] + misc[0, 5], 5.0)
        Vs = np.minimum(acc[64] + misc[0, 6], 5.0)
        tot = Vb + Vs + acc[32]
        G = np.minimum(np.maximum(-tot + misc[0, 7], 0.0), 5.0)
        out[f"out{j}"] = G.astype(np.float32)
    return out
